# revision 53
# baseline (speedup 1.0000x reference)
"""Trainium2 Bass kernel for nn_SAW_53395033424216 (grouped-covariance loss).

Math (see reference): for each sample b and channel-group g (16 channels),
  cov[b,g] = (Xg Xg^T)/(HW-1) with Xg rows scaled by wgh; loss is the
  mean-over-B sum-over-g of the masked (strict upper triangle) abs-sum of
  cov / num_off.

Strategy (measured 64.1us HW exec; baseline was 81.6us):
  * Host: compute perm/wgh from classifier_w (tiny), permute channels so each
    group is 16 consecutive channels, transpose each sample to [HW, 512] and
    cast to fp8e4 (rel err ~8.9e-4 against the 2e-2 gate; halves DMA bytes).
  * Device (8 cores, 2 samples each): stream 16 uniform 512KiB slabs per
    sample ([128hw x 8chunk x 512ch], 4KiB per partition line); for each
    128-channel block (= 8 whole groups) accumulate the 128x128 Gram over
    all 16384 hw rows with fp8 DoubleRow matmuls (K=256: one matmul per
    2-chunk pair, same tile as lhsT and rhs) -- 512 MMs/core at ~78ns warm.
    The stream is DMA-supply-paced (~410 GB/s, one HWDGE queue).
  * 34 warmup matmuls lift the PE HAM clock gate (4/8 -> 8/8) during the
    first slab's flight; any PE-idle gap >3.4us re-throttles to half clock.
  * Weight-scaling is bilinear -> folded into a per-block [128,128]
    mask/weight tile (loaded once via SWDGE so it takes no DMAHW lane slot),
    applied per sample with one DVE multiply + abs-reduce into bf16.
  * Cross-partition reduce on the PE (ones^T @ red) so the final store is a
    single 8-byte descriptor -- a [128,1] store's 128 4-byte descriptors
    take ~8.5us of HBM write receipts.
  * Host: sum the 8 cores' [1,2] outputs (clamp is a no-op: summands >= 0).

Synchronization: dpool bufs == 8 == the DMAHW lane round-robin period, and
16 slabs/sample keeps the phase across samples, so each slab DMA's tile-free
wait (consumers of slab k-8 done) provably subsumes Tile's same-lane
serialization wait; _reduce_sync_waits then reduces every instruction to the
single sync wait the hardware pseudo-instruction formats allow.  The pass
was verified race-free with CoreSim (bass_interp) -- its earlier version
wrongly dropped DMA same-lane waits ("own stream position") which races:
DMA issue order is not completion order.
"""

import os

# Whole-tile dependency tracking only: with per-subtile releases the slab DMA
# accumulates more sync-waits than the DMA pseudo-instruction format allows
# ("Too many sync wait commands" in walrus codegen).
os.environ.setdefault("BY_DEFAULT_DISABLE_SUBTILE_DEPS", "1")

import numpy as np
import ml_dtypes

import concourse.bass as bass
import concourse.mybir as mybir
from concourse.tile import TileContext
from concourse.bass_utils import run_bass_kernel_spmd

# Problem constants (hardcoded per the harness contract)
B = 16          # batch
CH = 512        # channels
H = W = 128
HW = H * W      # 16384
C = 16          # selected classes = group width
G = CH // C     # 32 groups
N_CORES = 8
SAMPLES_PER_CORE = B // N_CORES  # 2
NUM_OFF = C * (C - 1) // 2       # 120

# Data dtype on the wire/PE: bfloat16 (rel err ~2.4e-6) or float8_e4m3
# (rel err ~8.1e-4, half the DMA traffic).
DATA_DT_NAME = "float8e4"
SLAB = 8        # hw-chunks per DMA; 4 KiB per partition line (16 KiB tiles
                # run the PE at 94ns/MM vs 78 -- SBUF read-port conflicts)
N_WARMUP_MM = 34  # dummy matmuls during the initial DMA wait to lift the PE HAM throttle
DOUBLE_ROW = True  # fp8 DoubleRow: contract 2 hw-chunks per matmul (K=256)
N_CHUNKS = HW // 128             # 128
N_SLABS = N_CHUNKS // SLAB       # 16
N_CB = CH // 128                 # 4 channel blocks

_PROGRAM = None
LAST_RESULTS = None  # BassKernelResults of the most recent run (for test.py)


def _ensure_ntff_hook():
    """Provide antenv.axon_hooks if the image lacks it, so BASS_TRACE=1
    profiling works under axon (drives NTFF capture via the axon PJRT .so)."""
    try:
        import antenv.axon_hooks  # noqa: F401

        return
    except ImportError:
        pass
    import contextlib
    import ctypes
    import sys
    import types

    try:
        import antenv
    except ImportError:
        return

    so_path = "/opt/axon/libaxon_pjrt.so"
    if not os.path.exists(so_path):
        return
    lib = ctypes.CDLL(so_path)
    if not hasattr(lib, "axon_start_nrt_profile"):
        hook = None
    else:
        lib.axon_start_nrt_profile.argtypes = [
            ctypes.POINTER(ctypes.c_int64),
            ctypes.c_size_t,
        ]
        lib.axon_start_nrt_profile.restype = ctypes.c_int64
        lib.axon_stop_nrt_profile.argtypes = [ctypes.c_char_p]
        lib.axon_stop_nrt_profile.restype = ctypes.c_int64

        @contextlib.contextmanager
        def hook(output_dir, device_ids):
            import jax

            jax.devices()  # ensure the PJRT client exists before start
            if device_ids:
                ids = (ctypes.c_int64 * len(device_ids))(*device_ids)
                rc = lib.axon_start_nrt_profile(ids, len(device_ids))
            else:
                rc = lib.axon_start_nrt_profile(None, 0)
            if rc != 0:
                raise RuntimeError(f"axon_start_nrt_profile rc={rc}")
            try:
                yield
            finally:
                n = lib.axon_stop_nrt_profile(str(output_dir).encode())
                if n < 0:
                    raise RuntimeError(f"axon_stop_nrt_profile rc={n}")

    state = {"hook": hook}
    mod = types.ModuleType("antenv.axon_hooks")
    mod.get_axon_ntff_profile_hook = lambda: state["hook"]
    mod.set_axon_ntff_profile_hook = lambda h: state.update(hook=h)
    sys.modules["antenv.axon_hooks"] = mod
    antenv.axon_hooks = mod


_ensure_ntff_hook()


def _build_program():
    nc = bass.Bass()
    f32 = mybir.dt.float32
    data_dt = getattr(mybir.dt, DATA_DT_NAME)

    # Host pre-tiled layout: [s, slab, partition, k, c] so each partition's
    # slab slice is one contiguous 8 KiB run in DRAM (max DMA efficiency).
    xt = nc.dram_tensor(
        "xt", [SAMPLES_PER_CORE, N_SLABS, 128, SLAB, CH], data_dt, kind="ExternalInput"
    )
    wm = nc.dram_tensor("wm", [N_CB, 128, 128], f32, kind="ExternalInput")
    # Single-partition scalar output: a [128,1] output would emit 128 4-byte
    # DMA descriptors whose HBM write receipts crawl for ~8.5us at kernel end.
    out = nc.dram_tensor("out", [1, SAMPLES_PER_CORE], f32, kind="ExternalOutput")

    with TileContext(nc) as tc:
        with (
            tc.tile_pool(name="wpool", bufs=1) as wpool,
            tc.tile_pool(name="data", bufs=8) as dpool,
            tc.tile_pool(name="scratch", bufs=2) as spool,
            tc.tile_pool(name="redp", bufs=1) as redp,
            tc.tile_pool(name="psum", bufs=2, space="PSUM") as psum_pool,
        ):
            wm_t = wpool.tile([128, N_CB, 128], f32)

            bf16 = mybir.dt.bfloat16
            red_bf = redp.tile([128, SAMPLES_PER_CORE], bf16)

            # PE warm-up: ~3us of throwaway matmuls while the first data slab
            # is still in flight, so the HAM clock gate reaches 8/8 before the
            # real stream begins.  Shares the gram0 slot tag; real use of that
            # slot starts with start=True which clears it.
            # GpSimd's post-preamble phase starts ~1us before Vector's, so the
            # warmup matmuls (which only need warm_in) can begin earlier.
            # Narrow (N=128) warmups, GpSimd memset (its post-preamble phase
            # starts ~1us before Vector's for the small tile).  The periodic
            # 16KB queue-14 instruction reloads (each stalls the PE ~1.6us,
            # ~3 per run) persist even with zero GpSimd work -- they are PE
            # iram paging, unavoidable at this program size.
            warm_in = wpool.tile([128, 128], data_dt, name="warm_in")
            nc.gpsimd.memset(warm_in, 1)
            warm_ps = psum_pool.tile(
                [128, N_CB, 512], f32, name="warm_ps", tag="gram"
            )

            def warmup_mms(n):
                for _ in range(n):
                    nc.tensor.matmul(
                        warm_ps[:, 0, 0:128],
                        lhsT=warm_in,
                        rhs=warm_in,
                        start=True,
                        stop=True,
                    )

            warmup_mms(N_WARMUP_MM)

            # First slab of each sample is split into 2-chunk halves so the
            # first matmuls start as soon as 128 KiB has landed; after that the
            # PE (even cold at 428ns/chunk) never outruns the slab stream.
            # 16 uniform slabs per sample with bufs=8: tile k's pool-mate is
            # slab k-8, which is also its DMAHW lane predecessor (16 % 8 == 0
            # keeps the phase across samples), so the tile-free wait subsumes
            # Tile's same-lane serialization wait -- every DMA reduces to ONE
            # sync wait (the DMA pseudo-instruction format's hard limit).
            slab_plan = [(8 * sl, 8) for sl in range(N_SLABS)]

            # Single HWDGE queue (qSyncDynamicHW): measured best.  A dual
            # sync/scalar split (groups of 4, lane-pure) is sound with the
            # fixed reducer but ~3us slower -- two queue rows interleave
            # packets at the SDMA engines and fragment the HBM stream.
            def data_dma(out_ap, in_ap):
                nc.sync.dma_start(out=out_ap, in_=in_ap)

            # wm goes through SWDGE (gpsimd): it doesn't consume a DMAHW
            # lane slot, so every slab's lane phase stays uniform (tile k and
            # its pool-mate k-8 share a lane; the tile-free wait subsumes
            # Tile's same-lane serialization wait and the 1-wait reduction
            # stays provable), and it dispatches in parallel with the slabs
            # (wm-first on Sync delays slab0 and costs ~2.3us of ramp).
            nc.gpsimd.dma_start(out=wm_t, in_=wm.transpose([1, 0, 2]))
            # Tiny DVE read of wm_t: carries the SWDGE-completion wait early
            # on the Vector engine, so the later TTs' wm wait is provably
            # already satisfied (same-engine carried-wait rule above).
            wm_probe = spool.tile([1, 1], f32, name="wm_probe")
            nc.vector.tensor_copy(wm_probe, wm_t[0:1, 0, 0:1])

            for s in range(SAMPLES_PER_CORE):
                # One PSUM bank per channel-block Gram: a matmul's start=True
                # clears has_written for its WHOLE bank, so interleaved
                # accumulation groups must not share a bank.  Pad the per-cb
                # stride to 512 f32 (= one full bank); only cols 0:128 are used.
                gram = psum_pool.tile([128, N_CB, 512], f32, name="gram", tag="gram")
                for plan_i, (c0, csz) in enumerate(slab_plan):
                    dt_t = dpool.tile([128, SLAB, CH], data_dt)
                    src_ap = xt[s, c0 // SLAB]
                    if csz != SLAB:
                        src_ap = src_ap[:, c0 % SLAB : c0 % SLAB + csz]
                    data_dma(dt_t[:, :csz], src_ap)
                    if DOUBLE_ROW:
                        # fp8 DoubleRow: one matmul contracts a 2-chunk pair
                        # (K=256 via 2 fp8 weights per cell), halving the PE
                        # instruction stream.  lhsT and rhs use the same
                        # [128, 2, 128] slice of the slab tile.
                        for k in range(0, csz, 2):
                            h = c0 + k
                            for cb in range(N_CB):
                                t = dt_t[:, k : k + 2, cb * 128 : (cb + 1) * 128]
                                nc.tensor.matmul(
                                    gram[:, cb, 0:128],
                                    lhsT=t,
                                    rhs=t,
                                    start=(h == 0),
                                    stop=(h == N_CHUNKS - 2),
                                    perf_mode=mybir.MatmulPerfMode.DoubleRow,
                                )
                    else:
                        for k in range(csz):
                            h = c0 + k
                            for cb in range(N_CB):
                                t = dt_t[:, k, cb * 128 : (cb + 1) * 128]
                                nc.tensor.matmul(
                                    gram[:, cb, 0:128],
                                    lhsT=t,
                                    rhs=t,
                                    start=(h == 0),
                                    stop=(h == N_CHUNKS - 1),
                                )
                # post-process: red[:, s] = sum_{cb,j} |gram[i,cb,j]| * wm[cb,i,j]
                # (both ops run on Vector, so splitting buys nothing; keep one
                # full-size multiply + abs-reduce.  scr in bf16: summed over
                # 61k entries, per-entry rounding averages out.)
                scr = spool.tile([128, N_CB, 128], bf16)
                nc.vector.tensor_mul(scr, gram[:, :, 0:128], wm_t[:, :, :])
                with nc.allow_low_precision(
                    reason="abs-sum of 61k bf16 entries; rounding averages out"
                ):
                    nc.vector.tensor_reduce(
                        out=red_bf[:, s : s + 1],
                        in_=scr,
                        axis=mybir.AxisListType.XY,
                        op=mybir.AluOpType.add,
                        apply_absolute_value=True,
                    )

            # Cross-partition reduce on the PE (ones.T @ red_all -> [1, S]) so
            # the final store is ONE 8-byte descriptor instead of 128 4-byte
            # ones (whose completion receipts take ~8.5us).  bf16 operands:
            # the fp32 PE weight path is fragile in walrus, and bf16 rounding
            # of the two per-sample partials is ~2^-9 relative -- negligible.
            ones_t = wpool.tile([128, 1], bf16, name="ones_t")
            nc.vector.memset(ones_t, 1)
            final_ps = psum_pool.tile(
                [128, N_CB, 512], f32, name="final_ps", tag="gram"
            )
            nc.tensor.matmul(
                final_ps[0:1, 0, 0:SAMPLES_PER_CORE],
                lhsT=ones_t,
                rhs=red_bf,
                start=True,
                stop=True,
            )
            racc = spool.tile([1, SAMPLES_PER_CORE], f32, tag="racc")
            nc.vector.tensor_copy(racc, final_ps[0:1, 0, 0:SAMPLES_PER_CORE])
            # HWDGE store: the SWDGE (gpsimd) path costs a ~7us Pool-engine
            # drain at teardown; the Sync queue is idle by now.
            data_dma(out[:, :], racc)

    _reduce_sync_waits(nc)
    return nc


# Procs whose semaphores advance in instruction (program) order.  DMA lanes
# qualify: each lane's DMAs go through the same FIFO ring and complete (inc
# their lane sem) in issue order per SDMA engine.  GpSimd (Pool) does not
# (8 independent Q7 FIFOs) - we never emit Pool work.
_INORDER = ("PE", "DVE", "Activation", "SP", "DMAHW", "DMASW")


def _reduce_sync_waits(nc):
    """Walrus' per-instruction sync-wait capacity is 1 for DMA/compute
    pseudo-instructions (and small for Drain), but Tile's semaphore pass is
    not transitively minimal and can emit more. Reduce every wait list to
    its weakest sufficient single wait by proving the rest redundant:

    (a) a COMPUTE instruction's waits on its own engine sem are implied by
        stream position (engines execute sequentially).  NOT valid for DMA
        lane sems: dispatch is async, so issue order != completion order --
        Tile's same-lane serialization wait must survive (dropping it lets
        DMA #k+8 complete before #k and corrupts the lane count);
    (b) for each candidate kept wait (sem_k >= v_k): every other wait
        (sem_d >= v_d) must hold once sem_k reaches v_k.  That holds if an
        instruction at-or-before tick v_k in sem_k's stream carried
        (transitively) a wait implying it -- sems are monotone, so a wait
        that held once holds forever.
    """
    insts = [i for fn in nc.m.functions for blk in fn.blocks for i in blk.instructions]

    def proc_of_sem(name):
        return name.rsplit("_", 1)[0]  # e.g. "DMAHW3_44" -> "DMAHW3"

    # Per proc: ordered stream of (waits, cumulative-sem-value-after).
    streams = {}
    # Per instruction id: [(proc, sem-value-before-this-instruction)]
    positions = {}

    def add_to_stream(inst, proc, waits, upd):
        lst = streams.setdefault(proc, [])
        prev = lst[-1][1] if lst else 0
        positions.setdefault(id(inst), []).append((proc, prev))
        lst.append((waits, prev + upd))

    eng_sem = {"PE": "PE", "DVE": "DVE", "ACT": "Activation", "SP": "SP"}
    # Per instruction id: {sem: max wait value carried by EARLIER instructions
    # on the same engine} -- engines execute sequentially, so those waits have
    # already been satisfied when this instruction dispatches.
    eng_carried = {}
    eng_running: dict[str, dict[str, int]] = {}
    for inst in insts:
        si = inst.sync_info
        waits = [(w.ant_name, w.wait_value) for w in si.on_wait] if si else []
        en = str(inst.engine).split(".")[-1]
        run = eng_running.setdefault(en, {})
        eng_carried[id(inst)] = dict(run)
        for s_, v_ in waits:
            if run.get(s_, 0) < v_:
                run[s_] = v_
        if type(inst).__name__ == "InstDMACopy":
            # completion updates belong to the DMA lane proc
            for u in si.on_update:
                add_to_stream(inst, proc_of_sem(u.ant_name), waits, u.update_value)
        else:
            pref = eng_sem.get(en)
            if pref is None:
                continue
            upd = 0
            if si:
                for u in si.on_update:
                    if proc_of_sem(u.ant_name) == pref:
                        upd += u.update_value
            add_to_stream(inst, pref, waits, upd)

    from functools import lru_cache

    @lru_cache(maxsize=None)
    def holds(proc, tick, sem_d, v_d, depth=4):
        """Once `proc`'s sem has reached `tick`, does sem_d >= v_d hold?

        Covered prefix: entries up to the last one whose own completion is
        certified (cumulative sem value <= tick) have issued, so their waits
        held at some past moment; sems are monotone, so they hold now.
        """
        if proc == proc_of_sem(sem_d):
            return tick >= v_d
        if depth == 0 or not proc.startswith(_INORDER):
            return False
        stream = streams.get(proc, [])
        last = -1
        prev = 0
        for i, (waits, cum) in enumerate(stream):
            if cum > tick:
                break
            if cum > prev:
                last = i  # completing instruction within budget
            prev = cum
        for waits, _cum in stream[: last + 1]:
            for (s, v) in waits:
                if s == sem_d and v >= v_d:
                    return True
                if holds(proc_of_sem(s), v, sem_d, v_d, depth - 1):
                    return True
        return False

    splits: dict[int, list] = {}
    for inst in insts:
        tn = type(inst).__name__
        si = inst.sync_info
        if si is None or len(si.on_wait) <= 1:
            continue
        # Drop waits implied by the instruction's own position in its
        # in-order stream(s): at least `v` completions of that proc precede
        # it in program order.
        own = [
            (proc, prefix)
            for proc, prefix in positions.get(id(inst), [])
            if proc.startswith(("PE", "DVE", "Activation", "SP"))
        ]
        carried = eng_carried.get(id(inst), {})
        kept_sw = []
        for w in si.on_wait:
            wp = proc_of_sem(w.ant_name)
            if any(proc == wp and prefix >= w.wait_value for proc, prefix in own):
                continue
            if carried.get(w.ant_name, 0) >= w.wait_value:
                continue
            kept_sw.append(w)
        if len(kept_sw) <= 1:
            si.on_wait = kept_sw
            continue
        waits = [(w.ant_name, w.wait_value) for w in kept_sw]
        chosen = None
        for k, (sem_k, v_k) in enumerate(waits):
            if not proc_of_sem(sem_k).startswith(_INORDER):
                continue
            if all(
                holds(proc_of_sem(sem_k), v_k, sem_d, v_d)
                for d, (sem_d, v_d) in enumerate(waits)
                if d != k
            ):
                chosen = k
                break
        if chosen is None:
            # Same-lane serialization + tile-free on a DMA dispatch can be
            # genuinely independent, and the DMA pseudo-instruction format
            # only holds ONE wait ("Too many sync wait commands").  Split:
            # a NoOp on the same engine queue carries the extra waits -- the
            # engine FIFO blocks on it first, so the combined wait set is
            # unchanged.
            assert tn == "InstDMACopy", (
                f"{inst.name} ({tn}): cannot reduce waits to 1: {waits}"
            )
            splits.setdefault(id(inst), []).extend(kept_sw[:-1])
            si.on_wait = [kept_sw[-1]]
        else:
            si.on_wait = [kept_sw[chosen]]

    if splits:
        for fn in nc.m.functions:
            for blk in fn.blocks:
                out_insts = []
                for i in blk.instructions:
                    extra = splits.get(id(i))
                    if extra:
                        out_insts.append(
                            mybir.InstNoOp(
                                name=f"{i.name}-waitcarrier",
                                sync_info=mybir.SyncInfo(
                                    on_wait=extra, on_update=[]
                                ),
                                bass_nofuse=True,
                                engine=i.engine,
                            )
                        )
                    out_insts.append(i)
                blk.instructions = out_insts


def _host_prep(x, classifier_w, sel):
    """Compute perm / per-block weight-mask and the per-core bf16 shards."""
    x = np.asarray(x)
    w = np.asarray(classifier_w).astype(np.float32)
    sel = np.asarray(sel).astype(np.int64)

    w_abs = np.abs(w)
    idx = np.argsort(-w_abs, axis=1, kind="stable")  # matches jnp.argsort (stable)
    sig = (1.0 / (1.0 + np.exp(-w_abs.astype(np.float64)))).astype(np.float32)

    idx_sel = idx[sel]               # [C, CH]
    ch_ids = idx_sel[:, :G].T        # [G, C]
    perm = ch_ids.reshape(G * C)     # output channel g*C+c <- input channel
    wgh = sig[sel[None, :], ch_ids].reshape(G * C).astype(np.float64)

    # Per-channel-block weight/mask tile, with all scalar factors folded in:
    # wm[cb, i, j] = wgh_i * wgh_j * [same 16-group, j > i] / ((HW-1)*NUM_OFF*B)
    wm = np.zeros((N_CB, 128, 128), dtype=np.float64)
    scale = 1.0 / ((HW - 1) * NUM_OFF * B)
    ii, jj = np.meshgrid(np.arange(128), np.arange(128), indexing="ij")
    blockmask = ((ii // C) == (jj // C)) & (jj > ii)
    for cb in range(N_CB):
        wloc = wgh[cb * 128 : (cb + 1) * 128]
        wm[cb] = np.outer(wloc, wloc) * blockmask * scale
    wm = wm.astype(np.float32)

    # Per-core shards: samples [2c, 2c+1] -> permuted channels, hw-major,
    # pre-tiled as [s, slab, partition, k, c] so each partition's slab row is
    # one contiguous 8 KiB DRAM run.
    xr = x.reshape(B, CH, HW)
    shards = []
    for c in range(N_CORES):
        xs = xr[c * SAMPLES_PER_CORE : (c + 1) * SAMPLES_PER_CORE][:, perm, :]
        np_dt = mybir.dt.np(getattr(mybir.dt, DATA_DT_NAME))
        xb = xs.transpose(0, 2, 1).astype(np_dt)  # [S, HW, CH]
        xt = xb.reshape(SAMPLES_PER_CORE, N_SLABS, SLAB, 128, CH).transpose(
            0, 1, 3, 2, 4
        )
        # Per-core slab rotation: all 8 cores run the same program in near
        # lockstep, so without this they stream the same shard-relative
        # offsets simultaneously and beat against each other in HBM (the
        # supply-gap mean swings 80..100ns/MM run to run).  Rotating each
        # core's slab order decorrelates the streams; Gram accumulation is
        # order-independent.
        xt = np.ascontiguousarray(np.roll(xt, shift=-2 * c, axis=1))
        shards.append(xt)
    return shards, wm


def kernel(x, classifier_w, sel):
    global _PROGRAM, LAST_RESULTS
    assert x.shape == (B, CH, H, W), x.shape

    shards, wm = _host_prep(x, classifier_w, sel)

    if _PROGRAM is None:
        _PROGRAM = _build_program()

    in_maps = [{"xt": shards[c], "wm": wm} for c in range(N_CORES)]
    LAST_RESULTS = run_bass_kernel_spmd(_PROGRAM, in_maps, core_ids=list(range(N_CORES)))

    total = np.float64(0.0)
    for r in LAST_RESULTS.results:
        total += np.float64(r["out"].sum(dtype=np.float64))
    return np.array([total], dtype=np.float32)



# revision 54
# speedup vs baseline: 1.0381x; 1.0381x over previous
"""Trainium2 Bass kernel for nn_SAW_53395033424216 (grouped-covariance loss).

Math (see reference): for each sample b and channel-group g (16 channels),
  cov[b,g] = (Xg Xg^T)/(HW-1) with Xg rows scaled by wgh; loss is the
  mean-over-B sum-over-g of the masked (strict upper triangle) abs-sum of
  cov / num_off.

Strategy (measured 64.1us HW exec; baseline was 81.6us):
  * Host: compute perm/wgh from classifier_w (tiny), permute channels so each
    group is 16 consecutive channels, transpose each sample to [HW, 512] and
    cast to fp8e4 (rel err ~8.9e-4 against the 2e-2 gate; halves DMA bytes).
  * Device (8 cores, 2 samples each): stream 16 uniform 512KiB slabs per
    sample ([128hw x 8chunk x 512ch], 4KiB per partition line); for each
    128-channel block (= 8 whole groups) accumulate the 128x128 Gram over
    all 16384 hw rows with fp8 DoubleRow matmuls (K=256: one matmul per
    2-chunk pair, same tile as lhsT and rhs) -- 512 MMs/core at ~78ns warm.
    The stream is DMA-supply-paced (~410 GB/s, one HWDGE queue).
  * 34 warmup matmuls lift the PE HAM clock gate (4/8 -> 8/8) during the
    first slab's flight; any PE-idle gap >3.4us re-throttles to half clock.
  * Weight-scaling is bilinear -> folded into a per-block [128,128]
    mask/weight tile (loaded once via SWDGE so it takes no DMAHW lane slot),
    applied per sample with one DVE multiply + abs-reduce into bf16.
  * Cross-partition reduce on the PE (ones^T @ red) so the final store is a
    single 8-byte descriptor -- a [128,1] store's 128 4-byte descriptors
    take ~8.5us of HBM write receipts.
  * Host: sum the 8 cores' [1,2] outputs (clamp is a no-op: summands >= 0).

Synchronization: dpool bufs == 8 == the DMAHW lane round-robin period, and
16 slabs/sample keeps the phase across samples, so each slab DMA's tile-free
wait (consumers of slab k-8 done) provably subsumes Tile's same-lane
serialization wait; _reduce_sync_waits then reduces every instruction to the
single sync wait the hardware pseudo-instruction formats allow.  The pass
was verified race-free with CoreSim (bass_interp) -- its earlier version
wrongly dropped DMA same-lane waits ("own stream position") which races:
DMA issue order is not completion order.
"""

import os

# Whole-tile dependency tracking only: with per-subtile releases the slab DMA
# accumulates more sync-waits than the DMA pseudo-instruction format allows
# ("Too many sync wait commands" in walrus codegen).
os.environ.setdefault("BY_DEFAULT_DISABLE_SUBTILE_DEPS", "1")

import numpy as np
import ml_dtypes

import concourse.bass as bass
import concourse.mybir as mybir
from concourse.tile import TileContext
from concourse.bass_utils import run_bass_kernel_spmd

# Problem constants (hardcoded per the harness contract)
B = 16          # batch
CH = 512        # channels
H = W = 128
HW = H * W      # 16384
C = 16          # selected classes = group width
G = CH // C     # 32 groups
N_CORES = 8
SAMPLES_PER_CORE = B // N_CORES  # 2
NUM_OFF = C * (C - 1) // 2       # 120

# Data dtype on the wire/PE: bfloat16 (rel err ~2.4e-6) or float8_e4m3
# (rel err ~8.1e-4, half the DMA traffic).
DATA_DT_NAME = "float8e4"
SLAB = 8        # hw-chunks per DMA; 4 KiB per partition line (16 KiB tiles
                # run the PE at 94ns/MM vs 78 -- SBUF read-port conflicts)
N_WARMUP_MM = 34  # dummy matmuls during the initial DMA wait to lift the PE HAM throttle
DOUBLE_ROW = True  # fp8 DoubleRow: contract 2 hw-chunks per matmul (K=256)
N_CHUNKS = HW // 128             # 128
N_SLABS = N_CHUNKS // SLAB       # 16
N_CB = CH // 128                 # 4 channel blocks

_PROGRAM = None
LAST_RESULTS = None  # BassKernelResults of the most recent run (for test.py)


def _ensure_ntff_hook():
    """Provide antenv.axon_hooks if the image lacks it, so BASS_TRACE=1
    profiling works under axon (drives NTFF capture via the axon PJRT .so)."""
    try:
        import antenv.axon_hooks  # noqa: F401

        return
    except ImportError:
        pass
    import contextlib
    import ctypes
    import sys
    import types

    try:
        import antenv
    except ImportError:
        return

    so_path = "/opt/axon/libaxon_pjrt.so"
    if not os.path.exists(so_path):
        return
    lib = ctypes.CDLL(so_path)
    if not hasattr(lib, "axon_start_nrt_profile"):
        hook = None
    else:
        lib.axon_start_nrt_profile.argtypes = [
            ctypes.POINTER(ctypes.c_int64),
            ctypes.c_size_t,
        ]
        lib.axon_start_nrt_profile.restype = ctypes.c_int64
        lib.axon_stop_nrt_profile.argtypes = [ctypes.c_char_p]
        lib.axon_stop_nrt_profile.restype = ctypes.c_int64

        @contextlib.contextmanager
        def hook(output_dir, device_ids):
            import jax

            jax.devices()  # ensure the PJRT client exists before start
            if device_ids:
                ids = (ctypes.c_int64 * len(device_ids))(*device_ids)
                rc = lib.axon_start_nrt_profile(ids, len(device_ids))
            else:
                rc = lib.axon_start_nrt_profile(None, 0)
            if rc != 0:
                raise RuntimeError(f"axon_start_nrt_profile rc={rc}")
            try:
                yield
            finally:
                n = lib.axon_stop_nrt_profile(str(output_dir).encode())
                if n < 0:
                    raise RuntimeError(f"axon_stop_nrt_profile rc={n}")

    state = {"hook": hook}
    mod = types.ModuleType("antenv.axon_hooks")
    mod.get_axon_ntff_profile_hook = lambda: state["hook"]
    mod.set_axon_ntff_profile_hook = lambda h: state.update(hook=h)
    sys.modules["antenv.axon_hooks"] = mod
    antenv.axon_hooks = mod


_ensure_ntff_hook()


def _build_program():
    nc = bass.Bass()
    f32 = mybir.dt.float32
    data_dt = getattr(mybir.dt, DATA_DT_NAME)

    # Host pre-tiled layout: [s, slab, partition, k, c] so each partition's
    # slab slice is one contiguous 8 KiB run in DRAM (max DMA efficiency).
    xt = nc.dram_tensor(
        "xt", [SAMPLES_PER_CORE, N_SLABS, 128, SLAB, CH], data_dt, kind="ExternalInput"
    )
    wm = nc.dram_tensor("wm", [N_CB, 128, 128], f32, kind="ExternalInput")
    # Single-partition scalar output: a [128,1] output would emit 128 4-byte
    # DMA descriptors whose HBM write receipts crawl for ~8.5us at kernel end.
    out = nc.dram_tensor("out", [1, SAMPLES_PER_CORE], f32, kind="ExternalOutput")

    with TileContext(nc) as tc:
        with (
            tc.tile_pool(name="wpool", bufs=1) as wpool,
            tc.tile_pool(name="data", bufs=8) as dpool,
            tc.tile_pool(name="scratch", bufs=2) as spool,
            tc.tile_pool(name="redp", bufs=1) as redp,
            tc.tile_pool(name="psum", bufs=2, space="PSUM") as psum_pool,
        ):
            wm_t = wpool.tile([128, N_CB, 128], f32)

            bf16 = mybir.dt.bfloat16
            red_bf = redp.tile([128, SAMPLES_PER_CORE], bf16)

            # PE warm-up: ~3us of throwaway matmuls while the first data slab
            # is still in flight, so the HAM clock gate reaches 8/8 before the
            # real stream begins.  Shares the gram0 slot tag; real use of that
            # slot starts with start=True which clears it.
            # GpSimd's post-preamble phase starts ~1us before Vector's, so the
            # warmup matmuls (which only need warm_in) can begin earlier.
            # Narrow (N=128) warmups, GpSimd memset (its post-preamble phase
            # starts ~1us before Vector's for the small tile).  The periodic
            # 16KB queue-14 instruction reloads (each stalls the PE ~1.6us,
            # ~3 per run) persist even with zero GpSimd work -- they are PE
            # iram paging, unavoidable at this program size.
            warm_in = wpool.tile([128, 128], data_dt, name="warm_in")
            nc.gpsimd.memset(warm_in, 1)
            warm_ps = psum_pool.tile(
                [128, N_CB, 512], f32, name="warm_ps", tag="gram"
            )

            def warmup_mms(n):
                for _ in range(n):
                    nc.tensor.matmul(
                        warm_ps[:, 0, 0:128],
                        lhsT=warm_in,
                        rhs=warm_in,
                        start=True,
                        stop=True,
                    )

            warmup_mms(N_WARMUP_MM)

            # First slab of each sample is split into 2-chunk halves so the
            # first matmuls start as soon as 128 KiB has landed; after that the
            # PE (even cold at 428ns/chunk) never outruns the slab stream.
            # 16 uniform slabs per sample with bufs=8: tile k's pool-mate is
            # slab k-8, which is also its DMAHW lane predecessor (16 % 8 == 0
            # keeps the phase across samples), so the tile-free wait subsumes
            # Tile's same-lane serialization wait -- every DMA reduces to ONE
            # sync wait (the DMA pseudo-instruction format's hard limit).
            slab_plan = [(8 * sl, 8) for sl in range(N_SLABS)]

            # Single HWDGE queue (qSyncDynamicHW): measured best.  A dual
            # sync/scalar split (groups of 4, lane-pure) is sound with the
            # fixed reducer but ~3us slower -- two queue rows interleave
            # packets at the SDMA engines and fragment the HBM stream.
            def data_dma(out_ap, in_ap):
                nc.sync.dma_start(out=out_ap, in_=in_ap)

            # wm goes through SWDGE (gpsimd): it doesn't consume a DMAHW
            # lane slot, so every slab's lane phase stays uniform (tile k and
            # its pool-mate k-8 share a lane; the tile-free wait subsumes
            # Tile's same-lane serialization wait and the 1-wait reduction
            # stays provable), and it dispatches in parallel with the slabs
            # (wm-first on Sync delays slab0 and costs ~2.3us of ramp).
            nc.gpsimd.dma_start(out=wm_t, in_=wm.transpose([1, 0, 2]))
            # Tiny DVE read of wm_t: carries the SWDGE-completion wait early
            # on the Vector engine, so the later TTs' wm wait is provably
            # already satisfied (same-engine carried-wait rule above).
            wm_probe = spool.tile([1, 1], f32, name="wm_probe")
            nc.vector.tensor_copy(wm_probe, wm_t[0:1, 0, 0:1])

            for s in range(SAMPLES_PER_CORE):
                # One PSUM bank per channel-block Gram: a matmul's start=True
                # clears has_written for its WHOLE bank, so interleaved
                # accumulation groups must not share a bank.  Pad the per-cb
                # stride to 512 f32 (= one full bank); only cols 0:128 are used.
                gram = psum_pool.tile([128, N_CB, 512], f32, name="gram", tag="gram")
                for plan_i, (c0, csz) in enumerate(slab_plan):
                    dt_t = dpool.tile([128, SLAB, CH], data_dt)
                    src_ap = xt[s, c0 // SLAB]
                    if csz != SLAB:
                        src_ap = src_ap[:, c0 % SLAB : c0 % SLAB + csz]
                    data_dma(dt_t[:, :csz], src_ap)
                    if DOUBLE_ROW:
                        # fp8 DoubleRow: one matmul contracts a 2-chunk pair
                        # (K=256 via 2 fp8 weights per cell), halving the PE
                        # instruction stream.  lhsT and rhs use the same
                        # [128, 2, 128] slice of the slab tile.
                        for k in range(0, csz, 2):
                            h = c0 + k
                            for cb in range(N_CB):
                                t = dt_t[:, k : k + 2, cb * 128 : (cb + 1) * 128]
                                nc.tensor.matmul(
                                    gram[:, cb, 0:128],
                                    lhsT=t,
                                    rhs=t,
                                    start=(h == 0),
                                    stop=(h == N_CHUNKS - 2),
                                    perf_mode=mybir.MatmulPerfMode.DoubleRow,
                                )
                    else:
                        for k in range(csz):
                            h = c0 + k
                            for cb in range(N_CB):
                                t = dt_t[:, k, cb * 128 : (cb + 1) * 128]
                                nc.tensor.matmul(
                                    gram[:, cb, 0:128],
                                    lhsT=t,
                                    rhs=t,
                                    start=(h == 0),
                                    stop=(h == N_CHUNKS - 1),
                                )
                # post-process: red[:, s] = sum_{cb,j} |gram[i,cb,j]| * wm[cb,i,j]
                # (both ops run on Vector, so splitting buys nothing; keep one
                # full-size multiply + abs-reduce.  scr in bf16: summed over
                # 61k entries, per-entry rounding averages out.)
                scr = spool.tile([128, N_CB, 128], bf16)
                nc.vector.tensor_mul(scr, gram[:, :, 0:128], wm_t[:, :, :])
                with nc.allow_low_precision(
                    reason="abs-sum of 61k bf16 entries; rounding averages out"
                ):
                    nc.vector.tensor_reduce(
                        out=red_bf[:, s : s + 1],
                        in_=scr,
                        axis=mybir.AxisListType.XY,
                        op=mybir.AluOpType.add,
                        apply_absolute_value=True,
                    )

            # Cross-partition reduce on the PE (ones.T @ red_all -> [1, S]) so
            # the final store is ONE 8-byte descriptor instead of 128 4-byte
            # ones (whose completion receipts take ~8.5us).  bf16 operands:
            # the fp32 PE weight path is fragile in walrus, and bf16 rounding
            # of the two per-sample partials is ~2^-9 relative -- negligible.
            ones_t = wpool.tile([128, 1], bf16, name="ones_t")
            nc.vector.memset(ones_t, 1)
            final_ps = psum_pool.tile(
                [128, N_CB, 512], f32, name="final_ps", tag="gram"
            )
            nc.tensor.matmul(
                final_ps[0:1, 0, 0:SAMPLES_PER_CORE],
                lhsT=ones_t,
                rhs=red_bf,
                start=True,
                stop=True,
            )
            racc = spool.tile([1, SAMPLES_PER_CORE], f32, tag="racc")
            nc.vector.tensor_copy(racc, final_ps[0:1, 0, 0:SAMPLES_PER_CORE])
            # HWDGE store: the SWDGE (gpsimd) path costs a ~7us Pool-engine
            # drain at teardown; the Sync queue is idle by now.
            data_dma(out[:, :], racc)

    _reduce_sync_waits(nc)
    return nc


# Procs whose semaphores advance in instruction (program) order.  DMA lanes
# qualify: each lane's DMAs go through the same FIFO ring and complete (inc
# their lane sem) in issue order per SDMA engine.  GpSimd (Pool) does not
# (8 independent Q7 FIFOs) - we never emit Pool work.
_INORDER = ("PE", "DVE", "Activation", "SP", "DMAHW", "DMASW")


def _reduce_sync_waits(nc):
    """Walrus' per-instruction sync-wait capacity is 1 for DMA/compute
    pseudo-instructions (and small for Drain), but Tile's semaphore pass is
    not transitively minimal and can emit more. Reduce every wait list to
    its weakest sufficient single wait by proving the rest redundant:

    (a) a COMPUTE instruction's waits on its own engine sem are implied by
        stream position (engines execute sequentially).  NOT valid for DMA
        lane sems: dispatch is async, so issue order != completion order --
        Tile's same-lane serialization wait must survive (dropping it lets
        DMA #k+8 complete before #k and corrupts the lane count);
    (b) for each candidate kept wait (sem_k >= v_k): every other wait
        (sem_d >= v_d) must hold once sem_k reaches v_k.  That holds if an
        instruction at-or-before tick v_k in sem_k's stream carried
        (transitively) a wait implying it -- sems are monotone, so a wait
        that held once holds forever.
    """
    insts = [i for fn in nc.m.functions for blk in fn.blocks for i in blk.instructions]

    def proc_of_sem(name):
        return name.rsplit("_", 1)[0]  # e.g. "DMAHW3_44" -> "DMAHW3"

    # Per proc: ordered stream of (waits, cumulative-sem-value-after).
    streams = {}
    # Per instruction id: [(proc, sem-value-before-this-instruction)]
    positions = {}

    def add_to_stream(inst, proc, waits, upd):
        lst = streams.setdefault(proc, [])
        prev = lst[-1][1] if lst else 0
        positions.setdefault(id(inst), []).append((proc, prev))
        lst.append((waits, prev + upd))

    eng_sem = {"PE": "PE", "DVE": "DVE", "ACT": "Activation", "SP": "SP"}
    # Per instruction id: {sem: max wait value carried by EARLIER instructions
    # on the same engine} -- engines execute sequentially, so those waits have
    # already been satisfied when this instruction dispatches.
    eng_carried = {}
    eng_running: dict[str, dict[str, int]] = {}
    for inst in insts:
        si = inst.sync_info
        waits = [(w.ant_name, w.wait_value) for w in si.on_wait] if si else []
        en = str(inst.engine).split(".")[-1]
        run = eng_running.setdefault(en, {})
        eng_carried[id(inst)] = dict(run)
        for s_, v_ in waits:
            if run.get(s_, 0) < v_:
                run[s_] = v_
        if type(inst).__name__ == "InstDMACopy":
            # completion updates belong to the DMA lane proc
            for u in si.on_update:
                add_to_stream(inst, proc_of_sem(u.ant_name), waits, u.update_value)
        else:
            pref = eng_sem.get(en)
            if pref is None:
                continue
            upd = 0
            if si:
                for u in si.on_update:
                    if proc_of_sem(u.ant_name) == pref:
                        upd += u.update_value
            add_to_stream(inst, pref, waits, upd)

    from functools import lru_cache

    @lru_cache(maxsize=None)
    def holds(proc, tick, sem_d, v_d, depth=4):
        """Once `proc`'s sem has reached `tick`, does sem_d >= v_d hold?

        Covered prefix: entries up to the last one whose own completion is
        certified (cumulative sem value <= tick) have issued, so their waits
        held at some past moment; sems are monotone, so they hold now.
        """
        if proc == proc_of_sem(sem_d):
            return tick >= v_d
        if depth == 0 or not proc.startswith(_INORDER):
            return False
        stream = streams.get(proc, [])
        last = -1
        prev = 0
        for i, (waits, cum) in enumerate(stream):
            if cum > tick:
                break
            if cum > prev:
                last = i  # completing instruction within budget
            prev = cum
        for waits, _cum in stream[: last + 1]:
            for (s, v) in waits:
                if s == sem_d and v >= v_d:
                    return True
                if holds(proc_of_sem(s), v, sem_d, v_d, depth - 1):
                    return True
        return False

    splits: dict[int, list] = {}
    for inst in insts:
        tn = type(inst).__name__
        si = inst.sync_info
        if si is None or len(si.on_wait) <= 1:
            continue
        # Drop waits implied by the instruction's own position in its
        # in-order stream(s): at least `v` completions of that proc precede
        # it in program order.
        own = [
            (proc, prefix)
            for proc, prefix in positions.get(id(inst), [])
            if proc.startswith(("PE", "DVE", "Activation", "SP"))
        ]
        carried = eng_carried.get(id(inst), {})
        kept_sw = []
        for w in si.on_wait:
            wp = proc_of_sem(w.ant_name)
            if any(proc == wp and prefix >= w.wait_value for proc, prefix in own):
                continue
            if carried.get(w.ant_name, 0) >= w.wait_value:
                continue
            kept_sw.append(w)
        if len(kept_sw) <= 1:
            si.on_wait = kept_sw
            continue
        waits = [(w.ant_name, w.wait_value) for w in kept_sw]
        chosen = None
        for k, (sem_k, v_k) in enumerate(waits):
            if not proc_of_sem(sem_k).startswith(_INORDER):
                continue
            if all(
                holds(proc_of_sem(sem_k), v_k, sem_d, v_d)
                for d, (sem_d, v_d) in enumerate(waits)
                if d != k
            ):
                chosen = k
                break
        if chosen is None:
            # Same-lane serialization + tile-free on a DMA dispatch can be
            # genuinely independent, and the DMA pseudo-instruction format
            # only holds ONE wait ("Too many sync wait commands").  Split:
            # a NoOp on the same engine queue carries the extra waits -- the
            # engine FIFO blocks on it first, so the combined wait set is
            # unchanged.
            assert tn == "InstDMACopy", (
                f"{inst.name} ({tn}): cannot reduce waits to 1: {waits}"
            )
            splits.setdefault(id(inst), []).extend(kept_sw[:-1])
            si.on_wait = [kept_sw[-1]]
        else:
            si.on_wait = [kept_sw[chosen]]

    if splits:
        for fn in nc.m.functions:
            for blk in fn.blocks:
                out_insts = []
                for i in blk.instructions:
                    extra = splits.get(id(i))
                    if extra:
                        out_insts.append(
                            mybir.InstNoOp(
                                name=f"{i.name}-waitcarrier",
                                sync_info=mybir.SyncInfo(
                                    on_wait=extra, on_update=[]
                                ),
                                bass_nofuse=True,
                                engine=i.engine,
                            )
                        )
                    out_insts.append(i)
                blk.instructions = out_insts


def _host_prep(x, classifier_w, sel):
    """Compute perm / per-block weight-mask and the per-core bf16 shards."""
    x = np.asarray(x)
    w = np.asarray(classifier_w).astype(np.float32)
    sel = np.asarray(sel).astype(np.int64)

    w_abs = np.abs(w)
    idx = np.argsort(-w_abs, axis=1, kind="stable")  # matches jnp.argsort (stable)
    sig = (1.0 / (1.0 + np.exp(-w_abs.astype(np.float64)))).astype(np.float32)

    idx_sel = idx[sel]               # [C, CH]
    ch_ids = idx_sel[:, :G].T        # [G, C]
    perm = ch_ids.reshape(G * C)     # output channel g*C+c <- input channel
    wgh = sig[sel[None, :], ch_ids].reshape(G * C).astype(np.float64)

    # Per-channel-block weight/mask tile, with all scalar factors folded in:
    # wm[cb, i, j] = wgh_i * wgh_j * [same 16-group, j > i] / ((HW-1)*NUM_OFF*B)
    wm = np.zeros((N_CB, 128, 128), dtype=np.float64)
    scale = 1.0 / ((HW - 1) * NUM_OFF * B)
    ii, jj = np.meshgrid(np.arange(128), np.arange(128), indexing="ij")
    blockmask = ((ii // C) == (jj // C)) & (jj > ii)
    for cb in range(N_CB):
        wloc = wgh[cb * 128 : (cb + 1) * 128]
        wm[cb] = np.outer(wloc, wloc) * blockmask * scale
    wm = wm.astype(np.float32)

    # Per-core shards: samples [2c, 2c+1] -> permuted channels, hw-major,
    # pre-tiled as [s, slab, partition, k, c] so each partition's slab row is
    # one contiguous 8 KiB DRAM run.
    xr = x.reshape(B, CH, HW)
    shards = []
    for c in range(N_CORES):
        xs = xr[c * SAMPLES_PER_CORE : (c + 1) * SAMPLES_PER_CORE][:, perm, :]
        np_dt = mybir.dt.np(getattr(mybir.dt, DATA_DT_NAME))
        xb = xs.transpose(0, 2, 1).astype(np_dt)  # [S, HW, CH]
        # (A per-core slab rotation to decorrelate the 8 cores' HBM streams
        # was tried and showed no benefit -- the canonical in-order stream
        # measured best.)
        xt = np.ascontiguousarray(
            xb.reshape(SAMPLES_PER_CORE, N_SLABS, SLAB, 128, CH).transpose(
                0, 1, 3, 2, 4
            )
        )
        shards.append(xt)
    return shards, wm


def kernel(x, classifier_w, sel):
    global _PROGRAM, LAST_RESULTS
    assert x.shape == (B, CH, H, W), x.shape

    shards, wm = _host_prep(x, classifier_w, sel)

    if _PROGRAM is None:
        _PROGRAM = _build_program()

    in_maps = [{"xt": shards[c], "wm": wm} for c in range(N_CORES)]
    LAST_RESULTS = run_bass_kernel_spmd(_PROGRAM, in_maps, core_ids=list(range(N_CORES)))

    total = np.float64(0.0)
    for r in LAST_RESULTS.results:
        total += np.float64(r["out"].sum(dtype=np.float64))
    return np.array([total], dtype=np.float32)

